# revision 1
# baseline (speedup 1.0000x reference)
"""Multi-head attention (B=4, T=2048, C=1024, H=16, causal) on 8 TRN2 cores.

Sharding: core c -> batch b = c//2, head-half h2 = c%2 (8 heads / core).
Column-parallel QKV projections, per-core causal attention in transposed
layout, pairwise AllGather of head outputs, row-split output projection
(each core computes its T-half), host reassembles.
"""

import sys

sys.path.insert(0, "/opt/trn_rl_repo")

import numpy as np

import concourse.bacc as bacc
import concourse.bass as bass
import concourse.mybir as mybir
import concourse.tile as tile
from concourse.bass_utils import run_bass_kernel_spmd

F32 = mybir.dt.float32
F32R = mybir.dt.float32r
AF = mybir.ActivationFunctionType

P = 128          # partitions
T = 2048         # sequence length
C = 1024         # model dim
FS = 512         # per-core feature slice (8 heads x 64)
NH = 8           # heads per core
HD = 64          # head dim
THALF = 1024     # per-core output T slice
SCALE = 0.125    # 1/sqrt(64)
NCORES = 8

NTQ = 4          # T / 512 query tiles
NFB = 4          # FS / 128 feature blocks
NCB = 8          # C / 128 contraction blocks
NTT = 16         # T / 128 key tiles


def build_program():
    nc = bacc.Bacc(num_devices=NCORES)

    xq = nc.declare_dram_parameter("xq", [T, C], F32R, isOutput=False)
    xk = nc.declare_dram_parameter("xk", [T, C], F32R, isOutput=False)
    xv = nc.declare_dram_parameter("xv", [T, C], F32R, isOutput=False)
    # wq/wk[p, fb, cb, j] = W[128*cb + p, 512*h2 + 128*fb + j]
    wq = nc.declare_dram_parameter("wq", [P, NFB, NCB, P], F32R, isOutput=False)
    wk = nc.declare_dram_parameter("wk", [P, NFB, NCB, P], F32R, isOutput=False)
    wv = nc.declare_dram_parameter("wv", [C, FS], F32R, isOutput=False)
    # wo[p, cc, fc, j] = Wo[fsl, :][128*fc + p, 128*cc + j]
    wo = nc.declare_dram_parameter("wo", [P, NCB, NFB, P], F32R, isOutput=False)
    bq = nc.declare_dram_parameter("bq", [P, NFB], F32, isOutput=False)
    bk = nc.declare_dram_parameter("bk", [P, NFB], F32, isOutput=False)
    bv = nc.declare_dram_parameter("bv", [1, FS], F32, isOutput=False)
    bo = nc.declare_dram_parameter("bo", [P, NCB], F32, isOutput=False)
    ident = nc.declare_dram_parameter("ident", [P, P], F32R, isOutput=False)
    # maskx[p, u] = 1.0 iff u >= p + 384; diag-block i mask = maskx[:, 384-128i :][:512]
    maskx = nc.declare_dram_parameter("maskx", [P, 896], F32, isOutput=False)
    onesp = nc.declare_dram_parameter("onesp", [P, HD], F32R, isOutput=False)
    out = nc.declare_dram_parameter("out", [C, T], F32, isOutput=True)

    with tile.TileContext(nc) as tc:
        import contextlib

        with contextlib.ExitStack() as ctx:
            consts = ctx.enter_context(tc.tile_pool(name="consts", bufs=1))
            kt_pool = ctx.enter_context(tc.tile_pool(name="ktp", bufs=1))
            qt_pool = ctx.enter_context(tc.tile_pool(name="qtp", bufs=1))
            v_pool = ctx.enter_context(tc.tile_pool(name="vp", bufs=1))
            exp_pool = ctx.enter_context(tc.tile_pool(name="expp", bufs=4))
            expd_pool = ctx.enter_context(tc.tile_pool(name="expd", bufs=2))
            y_pool = ctx.enter_context(tc.tile_pool(name="yp", bufs=3))
            rc_pool = ctx.enter_context(tc.tile_pool(name="rcp", bufs=2))
            rb_pool = ctx.enter_context(tc.tile_pool(name="rbp", bufs=2))
            psA = ctx.enter_context(tc.tile_pool(name="psA", bufs=4, space="PSUM"))
            psB = ctx.enter_context(tc.tile_pool(name="psB", bufs=2, space="PSUM"))
            psY = ctx.enter_context(tc.tile_pool(name="psY", bufs=2, space="PSUM"))
            dram = ctx.enter_context(tc.tile_pool(name="dram", bufs=1, space="DRAM"))

            # ---- constants
            ones_sb = consts.tile([P, HD], F32R, tag="onesp", name="ones_sb")
            nc.sync.dma_start(ones_sb[:], onesp[:])
            ones64 = ones_sb[0:1, :]
            id_sb = consts.tile([P, P], F32R, tag="ident", name="id_sb")
            nc.sync.dma_start(id_sb[:], ident[:])
            mx_sb = consts.tile([P, 896], F32, tag="maskx", name="mx_sb")
            nc.sync.dma_start(mx_sb[:], maskx[:])
            bv_sb = consts.tile([P, FS], F32, tag="bv", name="bv_sb")
            nc.sync.dma_start(bv_sb[:], bv[:].to_broadcast((P, FS)))
            bq_t = consts.tile([P, NFB], F32, tag="bq", name="bq_t")
            nc.sync.dma_start(bq_t[:], bq[:])
            bk_t = consts.tile([P, NFB], F32, tag="bk", name="bk_t")
            nc.sync.dma_start(bk_t[:], bk[:])
            bo_t = consts.tile([P, NCB], F32, tag="bo", name="bo_t")
            nc.sync.dma_start(bo_t[:], bo[:])
            bq_sb = [bq_t[:, i : i + 1] for i in range(NFB)]
            bk_sb = [bk_t[:, i : i + 1] for i in range(NFB)]
            bo_sb = [bo_t[:, i : i + 1] for i in range(NCB)]

            # ---- persistent attention operands
            KT = [kt_pool.tile([P, T], F32R, tag=f"kt{i}", name=f"kt{i}")
                  for i in range(NFB)]
            QT = [qt_pool.tile([P, T], F32R, tag=f"qt{i}", name=f"qt{i}")
                  for i in range(NFB)]
            # V tiles carry an inline ones column per head: [v_h | 1] x 8
            VSB = [v_pool.tile([P, NH * (HD + 1)], F32R, tag=f"v{i}", name=f"v{i}")
                   for i in range(NTT)]

            # y^T staging; each core emits its partial out^T over full T and
            # the host sums the pair during unshard (bo passed as bo/2).
            y_part = dram.tile([FS, T], F32R, tag="y_part", name="y_part")

            # =====================  projections  =====================
            with contextlib.ExitStack() as pctx:
                xnat = pctx.enter_context(tc.tile_pool(name="xnat", bufs=3))
                xt_pool = pctx.enter_context(tc.tile_pool(name="xt", bufs=8))
                wsm = pctx.enter_context(tc.tile_pool(name="wsm", bufs=4))
                wbig = pctx.enter_context(tc.tile_pool(name="wbig", bufs=8))

                # K^T then Q^T: out[f, t] = sum_c W[c, f] X[t, c]
                for xin, wdram, bias_sb, OUT in (
                    (xk, wk, bk_sb, KT),
                    (xq, wq, bq_sb, QT),
                ):
                    wts = []
                    for fb in range(NFB):
                        wt = wsm.tile([P, NCB * P], F32R, tag="w", name="wt")
                        nc.sync.dma_start(
                            wt[:].rearrange("p (cb j) -> p cb j", j=P),
                            wdram[:, fb],
                        )
                        wts.append(wt)
                    for tq in range(NTQ):
                        # two DMAs per 512-row t-window (2 subtiles each)
                        xn2 = []
                        for hw_ in range(2):
                            xnh = xnat.tile([P, 2 * C], F32R, tag="xn", name="xn")
                            nc.sync.dma_start(
                                xnh[:].rearrange("p (tt c) -> p tt c", c=C),
                                xin[:].rearrange(
                                    "(w tt p) c -> w p tt c", p=P, tt=2
                                )[2 * tq + hw_],
                            )
                            xn2.append(xnh)
                        xtb = []
                        for cb in range(NCB):
                            ps = psB.tile([P, 512], F32R, tag="psB", name="ps_tr")
                            for tt in range(4):
                                nc.tensor.transpose(
                                    ps[:, P * tt : P * (tt + 1)],
                                    xn2[tt // 2][:, C * (tt % 2) + P * cb :
                                                 C * (tt % 2) + P * (cb + 1)],
                                    id_sb[:],
                                )
                            xt_t = xt_pool.tile([P, 512], F32R, tag="xt", name="xt_t")
                            nc.vector.tensor_copy(xt_t[:], ps[:])
                            xtb.append(xt_t)
                        for fb in range(NFB):
                            pp = psA.tile([P, 512], F32, tag="psA", name="pp")
                            for cb in range(NCB):
                                nc.tensor.matmul(
                                    pp[:], wts[fb][:, P * cb : P * (cb + 1)],
                                    xtb[cb][:],
                                    start=(cb == 0), stop=(cb == NCB - 1),
                                )
                            nc.vector.tensor_scalar_add(
                                OUT[fb][:, 512 * tq : 512 * (tq + 1)], pp[:],
                                bias_sb[fb],
                            )

                # V natural: out[t, f] = sum_c X[t, c] W[c, f]
                wv_sb = []
                for cb in range(NCB):
                    wvt = wbig.tile([P, FS], F32R, tag="wv", name="wvt")
                    nc.sync.dma_start(wvt[:], wv[P * cb : P * (cb + 1), :])
                    wv_sb.append(wvt)
                for ti in range(NTT):
                    if ti % 2 == 0:
                        xnv2 = xnat.tile([P, 2 * C], F32R, tag="xn", name="xnv")
                        nc.sync.dma_start(
                            xnv2[:].rearrange("p (tt c) -> p tt c", c=C),
                            xv[:].rearrange(
                                "(w tt p) c -> w p tt c", p=P, tt=2
                            )[ti // 2],
                        )
                    xn = xnv2[:, C * (ti % 2) : C * (ti % 2 + 1)]
                    xtv = []
                    for half in range(2):
                        ps = psB.tile([P, 512], F32R, tag="psB", name="ps_trv")
                        for j in range(4):
                            cb = 4 * half + j
                            nc.tensor.transpose(
                                ps[:, P * j : P * (j + 1)],
                                xn[:, P * cb : P * (cb + 1)],
                                id_sb[:],
                            )
                        xt_t = xt_pool.tile([P, 512], F32R, tag="xt", name="xtv_t")
                        nc.vector.tensor_copy(xt_t[:], ps[:])
                        xtv.append(xt_t)
                    pv = psA.tile([P, 512], F32, tag="psA", name="pv")
                    for cb in range(NCB):
                        lhsT = xtv[cb // 4][:, P * (cb % 4) : P * (cb % 4 + 1)]
                        nc.tensor.matmul(
                            pv[:], lhsT, wv_sb[cb][:],
                            start=(cb == 0), stop=(cb == NCB - 1),
                        )
                    vt = VSB[ti]
                    v3 = vt[:].rearrange("p (h x) -> p h x", x=HD + 1)
                    nc.vector.tensor_add(
                        v3[:, :, 0:HD],
                        pv[:].rearrange("p (h d) -> p h d", d=HD),
                        bv_sb[:].rearrange("p (h d) -> p h d", d=HD),
                    )
                    nc.vector.tensor_copy(v3[:, :, HD], ones_sb[:, 0:NH])

            # =====================  attention  =====================
            for pair in range(4):
                for tq in range(NTQ):
                    ntk = 4 * (tq + 1)
                    psy = [
                        psY.tile([HD + 1, 512], F32, tag="psY", name=f"psy{s}")
                        for s in range(2)
                    ]
                    qsl = slice(512 * tq, 512 * (tq + 1))

                    def s_mms(tk):
                        ksl = slice(P * tk, P * (tk + 1))
                        pss = []
                        for s in range(2):
                            rows = slice(64 * s, 64 * (s + 1))
                            ps = psA.tile([P, 512], F32, tag="psA", name=f"pss{s}")
                            nc.tensor.matmul(
                                ps[:], KT[pair][rows, ksl], QT[pair][rows, qsl],
                                start=True, stop=True,
                            )
                            pss.append(ps)
                        return pss

                    pss_next = s_mms(0)
                    for tk in range(ntk):
                        pss_cur = pss_next
                        exs = []
                        di = tk - 4 * tq
                        for s in range(2):
                            pool_ = expd_pool if 0 <= di <= 3 else exp_pool
                            tag_ = "expd" if 0 <= di <= 3 else "exp"
                            ex = pool_.tile([P, 512], F32R, tag=tag_, name="ex")
                            nc.scalar.activation(ex[:], pss_cur[s][:], AF.Exp,
                                                 scale=SCALE)
                            if 0 <= di <= 3:
                                off = 384 - 128 * di
                                w_ = P * (di + 1)
                                nc.vector.tensor_mul(
                                    ex[:, 0:w_], ex[:, 0:w_],
                                    mx_sb[:, off : off + w_]
                                )
                            exs.append(ex)
                        if tk + 1 < ntk:
                            pss_next = s_mms(tk + 1)
                        for s in range(2):
                            h = 2 * pair + s
                            vsl = slice((HD + 1) * h, (HD + 1) * (h + 1))
                            nc.tensor.matmul(
                                psy[s][:], VSB[tk][:, vsl], exs[s][:],
                                start=(tk == 0), stop=(tk == ntk - 1),
                            )
                    for s in range(2):
                        h = 2 * pair + s
                        rc = rc_pool.tile([1, 512], F32R, tag="rc", name="rc")
                        with nc.allow_low_precision(
                            reason="softmax recip row rounded to f32r for PE broadcast"
                        ):
                            nc.vector.reciprocal(rc[:], psy[s][HD : HD + 1, :])
                        # broadcast across partitions via ones ⊗ rc on the PE
                        rbp = psB.tile([HD, 512], F32, tag="psB", name="rbp")
                        nc.tensor.matmul(rbp[:], ones64, rc[:],
                                         start=True, stop=True)
                        rb = rb_pool.tile([HD, 512], F32, tag="rb", name="rb")
                        nc.vector.tensor_copy(rb[:], rbp[:])
                        ysb = y_pool.tile([HD, 512], F32R, tag="y", name="ysb")
                        nc.vector.tensor_mul(ysb[:], psy[s][0:HD, :], rb[:])
                        nc.sync.dma_start(
                            y_part[HD * h : HD * (h + 1), qsl], ysb[:]
                        )

            # ============  partial output projection + ReduceScatter  ============
            # partial_out^T[c, t] = sum_{f in my slice} Wo[f, c] y^T[f, t]
            with contextlib.ExitStack() as octx:
                ya_pool = octx.enter_context(tc.tile_pool(name="ya", bufs=4))
                wop = octx.enter_context(tc.tile_pool(name="wop", bufs=8))
                ob_pool = octx.enter_context(tc.tile_pool(name="ob", bufs=3))

                ych = []
                for fc in range(NFB):
                    yc = ya_pool.tile([P, T], F32R, tag="ya", name="yc")
                    nc.sync.dma_start(yc[:], y_part[P * fc : P * (fc + 1), :])
                    ych.append(yc)
                for cc in range(NCB):
                    wt = wop.tile([P, NFB * P], F32R, tag="wo", name="wo_t")
                    nc.sync.dma_start(
                        wt[:].rearrange("p (fc j) -> p fc j", j=P), wo[:, cc]
                    )
                    pso = [
                        psA.tile([P, 512], F32, tag="psA", name=f"pso{tt}")
                        for tt in range(NTQ)
                    ]
                    for fc in range(NFB):
                        for tt in range(NTQ):
                            nc.tensor.matmul(
                                pso[tt][:], wt[:, P * fc : P * (fc + 1)],
                                ych[fc][:, 512 * tt : 512 * (tt + 1)],
                                start=(fc == 0), stop=(fc == NFB - 1),
                            )
                    # host passes bo/2 so the host-side pair sum restores bo
                    osb = ob_pool.tile([P, 4 * 512], F32, tag="ob", name="osb")
                    for tt in range(NTQ):
                        nc.vector.tensor_scalar_add(
                            osb[:, 512 * tt : 512 * (tt + 1)], pso[tt][:],
                            bo_sb[cc])
                    nc.sync.dma_start(out[P * cc : P * (cc + 1), :], osb[:])


    nc.compile()
    return nc


_NC_CACHE = None


def _get_nc():
    global _NC_CACHE
    if _NC_CACHE is None:
        _NC_CACHE = build_program()
    return _NC_CACHE


def _host_consts():
    ident = np.eye(P, dtype=np.float32)
    pgrid, ugrid = np.mgrid[0:P, 0:896]
    maskxv = (ugrid >= pgrid + 384).astype(np.float32)
    onesv = np.ones((P, HD), dtype=np.float32)
    return ident, maskxv, onesv


def _w_qk_layout(w):
    # [p, fb, cb, j] = w[128*cb + p, 128*fb + j]
    return np.ascontiguousarray(
        w.reshape(NCB, P, NFB, P).transpose(1, 2, 0, 3))


def _w_o_layout(w):
    # [p, cc, fc, j] = w[128*fc + p, 128*cc + j]
    return np.ascontiguousarray(
        w.reshape(NFB, P, NCB, P).transpose(1, 2, 0, 3))


def _make_in_maps(inputs) -> list:
    q = np.asarray(inputs["q"], dtype=np.float32)
    k = np.asarray(inputs["k"], dtype=np.float32)
    v = np.asarray(inputs["v"], dtype=np.float32)
    Wq = np.asarray(inputs["Wq"], dtype=np.float32)
    Wk = np.asarray(inputs["Wk"], dtype=np.float32)
    Wv = np.asarray(inputs["Wv"], dtype=np.float32)
    Wo = np.asarray(inputs["Wo"], dtype=np.float32)
    bq = np.asarray(inputs["bq"], dtype=np.float32)
    bk = np.asarray(inputs["bk"], dtype=np.float32)
    bv = np.asarray(inputs["bv"], dtype=np.float32)
    bo = np.asarray(inputs["bo"], dtype=np.float32)
    # mask is all-ones in this problem (causal handled in-kernel); ignored.

    ident, maskxv, onesv = _host_consts()
    in_maps = []
    for c in range(NCORES):
        b, h2 = divmod(c, 2)
        fsl = slice(FS * h2, FS * (h2 + 1))
        in_maps.append({
            "xq": np.ascontiguousarray(q[b]),
            "xk": np.ascontiguousarray(k[b]),
            "xv": np.ascontiguousarray(v[b]),
            "wq": _w_qk_layout(Wq[:, fsl]),
            "wk": _w_qk_layout(Wk[:, fsl]),
            "wv": np.ascontiguousarray(Wv[:, fsl]),
            "wo": _w_o_layout(Wo[fsl, :]),
            "bq": np.ascontiguousarray(bq[fsl].reshape(NFB, P).T),
            "bk": np.ascontiguousarray(bk[fsl].reshape(NFB, P).T),
            "bv": np.ascontiguousarray(bv[fsl].reshape(1, FS)),
            "bo": np.ascontiguousarray((bo / 2.0).reshape(NCB, P).T),
            "ident": ident,
            "onesp": onesv,
            "maskx": maskxv,
        })
    return in_maps


def kernel(**inputs) -> np.ndarray:
    in_maps = _make_in_maps(inputs)
    nc = _get_nc()
    res = run_bass_kernel_spmd(nc, in_maps, list(range(NCORES)))

    full = np.empty((4, T, C), dtype=np.float32)
    for b in range(4):
        po = res.results[2 * b]["out"] + res.results[2 * b + 1]["out"]
        full[b] = po.T
    return full



# revision 21
# speedup vs baseline: 1.7294x; 1.7294x over previous
"""Multi-head attention (B=4, T=2048, C=1024, H=16, causal) on 8 TRN2 cores.

Sharding: core c -> batch b = c//2, head-half h2 = c%2 (8 heads / core).
v3: host-transposed bf16 X inputs (no PE transposes), bf16 matmuls
throughout, T-quarter streaming (V chunk -> K/Q chunks -> attention ->
output-projection chunk per 512-column window) so the scalar-engine exp
stream starts ~25us in and the output projection hides under it, diag
tiles narrowed to unmasked columns, softmax denominators batched at
partitions {0,32,64,96} for cheap reciprocals, 1/D broadcast via
stride-0 DMA, y kept in SBUF end to end.
"""

import sys

sys.path.insert(0, "/opt/trn_rl_repo")

import contextlib

import numpy as np

import concourse.bacc as bacc
import concourse.bass as bass
import concourse.mybir as mybir
import concourse.tile as tile
from concourse.bass_utils import run_bass_kernel_spmd

F32 = mybir.dt.float32
F32R = mybir.dt.float32r
BF16 = mybir.dt.bfloat16
AF = mybir.ActivationFunctionType

P = 128          # partitions
T = 2048         # sequence length
C = 1024         # model dim
FS = 512         # per-core feature slice (8 heads x 64)
NH = 8           # heads per core
HD = 64          # head dim
SCALE = 0.125    # 1/sqrt(64)
NCORES = 8

NTQ = 4          # T / 512 query windows (quarters)
NFB = 4          # FS / 128 feature blocks (head pairs)
NCB = 8          # C / 128 contraction blocks
NTT = 16         # T / 128 key tiles


def build_program():
    nc = bacc.Bacc(num_devices=NCORES)

    xqT = nc.declare_dram_parameter("xqT", [C, T], BF16, isOutput=False)
    xkT = nc.declare_dram_parameter("xkT", [C, T], BF16, isOutput=False)
    xvT = nc.declare_dram_parameter("xvT", [C, T], BF16, isOutput=False)
    # wq/wk[p, cb, 128*fb + j] = W[128*cb + p, 512*h2 + 128*fb + j]
    wq = nc.declare_dram_parameter("wq", [P, NCB, FS], BF16, isOutput=False)
    wk = nc.declare_dram_parameter("wk", [P, NCB, FS], BF16, isOutput=False)
    wv = nc.declare_dram_parameter("wv", [C, FS], BF16, isOutput=False)
    # wo[p, cc, 128*fc + j] = Wo[fsl, :][128*fc + p, 128*cc + j]
    wo = nc.declare_dram_parameter("wo", [P, NCB, NFB * P], BF16, isOutput=False)
    bq = nc.declare_dram_parameter("bq", [P, NFB], F32, isOutput=False)
    bk = nc.declare_dram_parameter("bk", [P, NFB], F32, isOutput=False)
    bv = nc.declare_dram_parameter("bv", [1, FS], F32, isOutput=False)
    bo = nc.declare_dram_parameter("bo", [P, NCB], F32, isOutput=False)
    # maskc[p, u] = 1.0 iff u >= p (upper triangular incl diagonal)
    maskc = nc.declare_dram_parameter("maskc", [P, P], BF16, isOutput=False)
    onesb = nc.declare_dram_parameter("onesb", [P, NH], BF16, isOutput=False)
    out = nc.declare_dram_parameter("out", [C, T], F32, isOutput=True)

    with tile.TileContext(nc) as tc:
        with contextlib.ExitStack() as ctx:
            consts = ctx.enter_context(tc.tile_pool(name="consts", bufs=1))
            kt_pool = ctx.enter_context(tc.tile_pool(name="ktp", bufs=1))
            qt_pool = ctx.enter_context(tc.tile_pool(name="qtp", bufs=1))
            v_pool = ctx.enter_context(tc.tile_pool(name="vp", bufs=1))
            ya_pool = ctx.enter_context(tc.tile_pool(name="yap", bufs=1))
            exp_pool = ctx.enter_context(tc.tile_pool(name="expp", bufs=6))
            w_pool = ctx.enter_context(tc.tile_pool(name="wp", bufs=1))
            xv_pool = ctx.enter_context(tc.tile_pool(name="xvp", bufs=16))
            xk_pool = ctx.enter_context(tc.tile_pool(name="xkp", bufs=16))
            xq_pool = ctx.enter_context(tc.tile_pool(name="xqp", bufs=16))
            d_pool = ctx.enter_context(tc.tile_pool(name="dp", bufs=4))
            rb_pool = ctx.enter_context(tc.tile_pool(name="rbp", bufs=4))
            ob_pool = ctx.enter_context(tc.tile_pool(name="obp", bufs=3))
            psP = ctx.enter_context(tc.tile_pool(name="psP", bufs=2, space="PSUM"))
            psA = ctx.enter_context(tc.tile_pool(name="psA", bufs=4, space="PSUM"))
            psY = ctx.enter_context(tc.tile_pool(name="psY", bufs=2, space="PSUM"))
            dram = ctx.enter_context(tc.tile_pool(name="dram", bufs=2,
                                                  space="DRAM"))

            # ---- small constants
            onesb_sb = consts.tile([P, NH], BF16, tag="onesb", name="onesb_sb")
            nc.sync.dma_start(onesb_sb[:], onesb[:])
            mx_sb = consts.tile([P, P], BF16, tag="maskc", name="mx_sb")
            nc.sync.dma_start(mx_sb[:], maskc[:])
            bv_sb = consts.tile([P, FS], F32, tag="bv", name="bv_sb")
            nc.sync.dma_start(bv_sb[:], bv[:].to_broadcast((P, FS)))
            bq_t = consts.tile([P, NFB], F32, tag="bq", name="bq_t")
            nc.sync.dma_start(bq_t[:], bq[:])
            bk_t = consts.tile([P, NFB], F32, tag="bk", name="bk_t")
            nc.sync.dma_start(bk_t[:], bk[:])
            bo_t = consts.tile([P, NCB], F32, tag="bo", name="bo_t")
            nc.sync.dma_start(bo_t[:], bo[:])
            bq_sb = [bq_t[:, i : i + 1] for i in range(NFB)]
            bk_sb = [bk_t[:, i : i + 1] for i in range(NFB)]
            bo_sb = [bo_t[:, i : i + 1] for i in range(NCB)]

            # ---- weights (emitted in first-use order for the DMA queue)
            wv_sb = [w_pool.tile([P, FS], BF16, tag=f"wv{cb}", name=f"wv{cb}")
                     for cb in range(NCB)]
            for cb in range(NCB):
                nc.sync.dma_start(wv_sb[cb][:], wv[P * cb : P * (cb + 1), :])
            wq_sb = w_pool.tile([P, NCB * FS], BF16, tag="wq", name="wq_sb")
            wk_sb = w_pool.tile([P, NCB * FS], BF16, tag="wk", name="wk_sb")
            wo_sb = w_pool.tile([P, NCB * NFB * P], BF16, tag="wo", name="wo_sb")

            # ---- persistent attention operands
            KT = [kt_pool.tile([P, T], BF16, tag=f"kt{i}", name=f"kt{i}")
                  for i in range(NFB)]
            QT = [qt_pool.tile([P, T], BF16, tag=f"qt{i}", name=f"qt{i}")
                  for i in range(NFB)]
            # V tiles carry an inline ones column per head: [v_h | 1] x 8
            VSB = [v_pool.tile([P, NH * (HD + 1)], BF16, tag=f"v{i}", name=f"v{i}")
                   for i in range(NTT)]
            # y^T per pair: heads (2p, 2p+1) at partition 0/64, all T columns
            YA = [ya_pool.tile([P, T], BF16, tag=f"ya{i}", name=f"ya{i}")
                  for i in range(NFB)]

            first_w = True
            for w in range(NTQ):
                wsl = slice(FS * w, FS * (w + 1))
                # ---- X chunk DMAs for this quarter
                xv_t, xk_t, xq_t = [], [], []
                for cb in range(NCB):
                    t_ = xv_pool.tile([P, FS], BF16, tag="xv", name=f"xv{cb}_{w}")
                    nc.sync.dma_start(t_[:], xvT[P * cb : P * (cb + 1), wsl])
                    xv_t.append(t_)
                if first_w:
                    nc.sync.dma_start(
                        wq_sb[:].rearrange("p (cb j) -> p cb j", j=FS), wq[:])
                    nc.sync.dma_start(
                        wk_sb[:].rearrange("p (cb j) -> p cb j", j=FS), wk[:])
                for cb in range(NCB):
                    t_ = xk_pool.tile([P, FS], BF16, tag="xk", name=f"xk{cb}_{w}")
                    nc.sync.dma_start(t_[:], xkT[P * cb : P * (cb + 1), wsl])
                    xk_t.append(t_)
                for cb in range(NCB):
                    t_ = xq_pool.tile([P, FS], BF16, tag="xq", name=f"xq{cb}_{w}")
                    nc.sync.dma_start(t_[:], xqT[P * cb : P * (cb + 1), wsl])
                    xq_t.append(t_)
                if first_w:
                    nc.sync.dma_start(
                        wo_sb[:].rearrange("p (cc j) -> p cc j", j=NFB * P),
                        wo[:])
                    first_w = False

                # ---- V projection for this quarter's 4 key tiles
                for ti in range(4 * w, 4 * w + 4):
                    pv = psP.tile([P, FS], F32, tag="psP", name="pv")
                    for cb in range(NCB):
                        nc.tensor.matmul(
                            pv[:],
                            xv_t[cb][:, P * (ti % 4) : P * (ti % 4 + 1)],
                            wv_sb[cb][:],
                            start=(cb == 0), stop=(cb == NCB - 1),
                        )
                    vt = VSB[ti]
                    v3 = vt[:].rearrange("p (h x) -> p h x", x=HD + 1)
                    nc.vector.tensor_add(
                        v3[:, :, 0:HD],
                        pv[:].rearrange("p (h d) -> p h d", d=HD),
                        bv_sb[:].rearrange("p (h d) -> p h d", d=HD),
                    )
                    nc.vector.tensor_copy(v3[:, :, HD], onesb_sb[:])

                # ---- K/Q projection chunks for every pair
                for pair in range(NFB):
                    for wsb, xt_, bias_sb, OUT in (
                        (wk_sb, xk_t, bk_sb, KT),
                        (wq_sb, xq_t, bq_sb, QT),
                    ):
                        pp = psP.tile([P, FS], F32, tag="psP", name="pp")
                        for cb in range(NCB):
                            nc.tensor.matmul(
                                pp[:],
                                wsb[:, FS * cb + P * pair :
                                    FS * cb + P * (pair + 1)],
                                xt_[cb][:],
                                start=(cb == 0), stop=(cb == NCB - 1),
                            )
                        nc.vector.tensor_scalar_add(
                            OUT[pair][:, wsl], pp[:], bias_sb[pair],
                        )

                # ---- denominator staging tiles for this quarter
                dts = [d_pool.tile([P, FS], F32, tag="dt", name=f"dt{a}_{w}")
                       for a in range(2)]
                for a in range(2):
                    nc.vector.memset(dts[a][:], 1.0)

                # ---- attention (pair, tq=w) for all pairs
                ntk = 4 * (w + 1)
                for pair in range(NFB):
                    psy = [psY.tile([HD + 1, FS], F32, tag="psY",
                                    name=f"psy{s}") for s in range(2)]

                    def s_mms(tk):
                        di = tk - 4 * w
                        off = P * di if di >= 0 else 0
                        ksl = slice(P * tk, P * (tk + 1))
                        pss = []
                        for s in range(2):
                            rows = slice(HD * s, HD * (s + 1))
                            ps = psA.tile([P, FS], F32, tag="psA",
                                          name=f"pss{s}")
                            nc.tensor.matmul(
                                ps[:, off:FS],
                                KT[pair][rows, ksl],
                                QT[pair][rows, FS * w + off : FS * (w + 1)],
                                start=True, stop=True,
                            )
                            pss.append(ps)
                        return pss

                    pss_next = s_mms(0)
                    for tk in range(ntk):
                        pss_cur = pss_next
                        di = tk - 4 * w
                        off = P * di if di >= 0 else 0
                        exs = []
                        for s in range(2):
                            ex = exp_pool.tile([P, FS], BF16, tag="exp",
                                               name="ex")
                            nc.scalar.activation(
                                ex[:, off:FS], pss_cur[s][:, off:FS],
                                AF.Exp, scale=SCALE,
                            )
                            if di >= 0:
                                # triangular boundary block: fixed 128 cols
                                nc.vector.tensor_mul(
                                    ex[:, off : off + P],
                                    ex[:, off : off + P],
                                    mx_sb[:],
                                )
                            exs.append(ex)
                        if tk + 1 < ntk:
                            pss_next = s_mms(tk + 1)
                        for s in range(2):
                            h = 2 * pair + s
                            vsl = slice((HD + 1) * h, (HD + 1) * (h + 1))
                            nc.tensor.matmul(
                                psy[s][:, off:FS], VSB[tk][:, vsl],
                                exs[s][:, off:FS],
                                start=(tk == 0), stop=(tk == ntk - 1),
                            )
                    # stash unnormalized y and the denominator row
                    for s in range(2):
                        m = 2 * pair + s
                        nc.vector.tensor_copy(
                            YA[pair][HD * s : HD * (s + 1), wsl],
                            psy[s][0:HD, :],
                        )
                        nc.vector.tensor_copy(
                            dts[m // 4][32 * (m % 4) : 32 * (m % 4) + 1, :],
                            psy[s][HD : HD + 1, :],
                        )

                # ---- batched reciprocals + DMA-broadcast normalize
                rts = [d_pool.tile([P, FS], F32, tag="rt", name=f"rt{a}_{w}")
                       for a in range(2)]
                rdram = dram.tile([2 * NFB, FS], F32, tag="rd", name=f"rd{w}")
                for a in range(2):
                    nc.vector.reciprocal(rts[a][:], dts[a][:])
                for pair in range(NFB):
                    for s in range(2):
                        m = 2 * pair + s
                        nc.sync.dma_start(
                            rdram[m : m + 1, :],
                            rts[m // 4][32 * (m % 4) : 32 * (m % 4) + 1, :],
                        )
                for pair in range(NFB):
                    rb = rb_pool.tile([P, FS], F32, tag="rb", name="rb")
                    for s in range(2):
                        m = 2 * pair + s
                        rbs = rb[HD * s : HD * (s + 1), :]
                        nc.sync.dma_start(
                            rbs, rdram[m : m + 1, :].to_broadcast((HD, FS)),
                        )
                        ysl = YA[pair][HD * s : HD * (s + 1), wsl]
                        nc.vector.tensor_mul(ysl, ysl, rbs)

                # ---- output projection chunk for this quarter
                for cc in range(NCB):
                    pso = psA.tile([P, FS], F32, tag="psA", name="pso")
                    for fc in range(NFB):
                        lhsT = wo_sb[:, NFB * P * cc + P * fc :
                                     NFB * P * cc + P * (fc + 1)]
                        nc.tensor.matmul(
                            pso[:], lhsT, YA[fc][:, wsl],
                            start=(fc == 0), stop=(fc == NFB - 1),
                        )
                    # host passes bo/2 so the host-side pair sum restores bo
                    osb = ob_pool.tile([P, FS], F32, tag="ob", name="osb")
                    nc.vector.tensor_scalar_add(osb[:], pso[:], bo_sb[cc])
                    nc.sync.dma_start(out[P * cc : P * (cc + 1), wsl], osb[:])

    nc.compile()
    return nc


_NC_CACHE = None


def _get_nc():
    global _NC_CACHE
    if _NC_CACHE is None:
        _NC_CACHE = build_program()
    return _NC_CACHE


def _make_in_maps(inputs) -> list:
    import ml_dtypes

    bf16 = ml_dtypes.bfloat16
    q = np.asarray(inputs["q"], dtype=np.float32)
    k = np.asarray(inputs["k"], dtype=np.float32)
    v = np.asarray(inputs["v"], dtype=np.float32)
    Wq = np.asarray(inputs["Wq"], dtype=np.float32)
    Wk = np.asarray(inputs["Wk"], dtype=np.float32)
    Wv = np.asarray(inputs["Wv"], dtype=np.float32)
    Wo = np.asarray(inputs["Wo"], dtype=np.float32)
    bq = np.asarray(inputs["bq"], dtype=np.float32)
    bk = np.asarray(inputs["bk"], dtype=np.float32)
    bv = np.asarray(inputs["bv"], dtype=np.float32)
    bo = np.asarray(inputs["bo"], dtype=np.float32)
    # mask is all-ones in this problem (causal handled in-kernel); ignored.

    pgrid, ugrid = np.mgrid[0:P, 0:P]
    maskcv = (ugrid >= pgrid).astype(bf16)
    onesbv = np.ones((P, NH), dtype=bf16)

    def _w_qk(w):
        # [p, cb, j] = w[128*cb + p, j]   (w is the [C, FS] slice)
        return np.ascontiguousarray(
            w.reshape(NCB, P, FS).transpose(1, 0, 2)).astype(bf16)

    def _w_o(w):
        # [p, cc, 128*fc + j] = w[128*fc + p, 128*cc + j]  (w is [FS, C])
        return np.ascontiguousarray(
            w.reshape(NFB, P, NCB, P).transpose(1, 2, 0, 3)
             .reshape(P, NCB, NFB * P)).astype(bf16)

    in_maps = []
    for c in range(NCORES):
        b, h2 = divmod(c, 2)
        fsl = slice(FS * h2, FS * (h2 + 1))
        in_maps.append({
            "xqT": np.ascontiguousarray(q[b].T).astype(bf16),
            "xkT": np.ascontiguousarray(k[b].T).astype(bf16),
            "xvT": np.ascontiguousarray(v[b].T).astype(bf16),
            "wq": _w_qk(Wq[:, fsl]),
            "wk": _w_qk(Wk[:, fsl]),
            "wv": np.ascontiguousarray(Wv[:, fsl]).astype(bf16),
            "wo": _w_o(Wo[fsl, :]),
            "bq": np.ascontiguousarray(bq[fsl].reshape(NFB, P).T),
            "bk": np.ascontiguousarray(bk[fsl].reshape(NFB, P).T),
            "bv": np.ascontiguousarray(bv[fsl].reshape(1, FS)),
            "bo": np.ascontiguousarray((bo / 2.0).reshape(NCB, P).T),
            "onesb": onesbv,
            "maskc": maskcv,
        })
    return in_maps


def kernel(**inputs) -> np.ndarray:
    in_maps = _make_in_maps(inputs)
    nc = _get_nc()
    res = run_bass_kernel_spmd(nc, in_maps, list(range(NCORES)))

    full = np.empty((4, T, C), dtype=np.float32)
    for b in range(4):
        po = res.results[2 * b]["out"] + res.results[2 * b + 1]["out"]
        full[b] = po.T
    return full
